# revision 7
# baseline (speedup 1.0000x reference)
"""Trainium2 Bass kernel for nn_BackboneModel (backbone frame rebuild).

The reference scatters rows into a padded [B, L, 14, 3] block, builds
Gram-Schmidt rigid frames from (N, CA, C), places ideal N/CA/C/O atoms,
and gathers the valid rows back.  Scatter followed by gather at the same
(batch_id, pos) indices is an identity permutation over the valid rows,
so the whole model is a pure per-row function of X[i]:

    e1 = normalize(C - CA)                      (normalize: v * rsqrt(|v|^2 + eps^2))
    e2 = normalize((N - CA) - ((N - CA).e1) e1)
    out[0] = -0.525*e1 + 1.363*e2 + CA          (N)
    out[1] = CA                                 (CA)
    out[2] =  1.526*e1            + CA          (C)
    out[3] =  2.153*e1 - 1.062*e2 + CA          (O)
    out[4:14] = X[4:14]                         (passthrough)

(X_IDEAL has z == 0 for all four atoms, so e3 = e1 x e2 is never needed,
and batch_ids never affects output values.)

Numerics: the Gram-Schmidt rejection w = v - (v.e1)e1 suffers catastrophic
cancellation, which amplifies any error in e1 by ~|v|/|w| (observed 250x).
The ACT-engine Sqrt is table-based (~7e-6 rel), so e1 via sqrt+reciprocal
is not accurate enough for that path.  Instead the rejection uses the exact
DVE reciprocal:  w = v - ((v.d1) / (|d1|^2 + eps^2)) d1,  and the table
sqrt is only used for the final normalize scalars, where its error is not
amplified.  Measured absmax vs the f32 jax reference: ~5e-5.

Sharding: data-parallel, 8 equal contiguous row chunks of 98304 rows.
Each core processes its chunk as 6 tiles of [128 partitions x 128 rows x 42 f32],
computing in place in the loaded tile so both the load and the store are a
single fully-contiguous ~2.75 MB DMA per tile.
"""

import numpy as np

N_CORES = 8
N_TOTAL = 786432
N_CORE = N_TOTAL // N_CORES      # 98304 rows per core
P = 128                          # SBUF partitions
R = 128                          # rows per partition per tile
ROWS_PER_TILE = P * R            # 16384
TILES = N_CORE // ROWS_PER_TILE  # 6
C42 = 42                         # 14 atoms * 3 coords
EPS2 = 1e-6                      # FrameBuilder distance_eps squared

_NC = None


def _build_nc():
    import concourse.bacc as bacc
    import concourse.tile as tile
    from concourse import mybir

    f32 = mybir.dt.float32
    AX = mybir.AxisListType.X
    MUL = mybir.AluOpType.mult
    ADD = mybir.AluOpType.add
    SQRT = mybir.ActivationFunctionType.Sqrt
    SQUARE = mybir.ActivationFunctionType.Square
    IDENT = mybir.ActivationFunctionType.Identity

    nc = bacc.Bacc()
    X = nc.declare_dram_parameter("X", [TILES, P, R, C42], f32, isOutput=False)
    Y = nc.declare_dram_parameter("Y", [TILES, P, R, C42], f32, isOutput=True)

    def bcast(s):  # [P, R] per-row scalar -> [P, R, 3]
        return s[:, :, None].broadcast_to([P, R, 3])

    with tile.TileContext(nc) as tc:
        with tc.tile_pool(name="io", bufs=3) as io, \
             tc.tile_pool(name="v3", bufs=2) as v3, \
             tc.tile_pool(name="sc", bufs=2) as sc, \
             tc.tile_pool(name="one", bufs=1) as one:
            eps = one.tile([P, 1], f32)
            nc.vector.memset(eps, EPS2)
            zero = one.tile([P, 1], f32)
            nc.vector.memset(zero, 0.0)

            for i in range(TILES):
                T = io.tile([P, R, C42], f32)
                nc.sync.dma_start(out=T, in_=X[i])

                Na = T[:, :, 0:3]    # N  (input; overwritten with out_N)
                CAa = T[:, :, 3:6]   # CA (unchanged -> out_CA)
                Ca = T[:, :, 6:9]    # C  (input; overwritten with out_C)
                Oa = T[:, :, 9:12]   # O  (input unused; overwritten with out_O)

                D1 = v3.tile([P, R, 3], f32)
                V = v3.tile([P, R, 3], f32)
                SQ = v3.tile([P, R, 3], f32)
                P2 = v3.tile([P, R, 3], f32)
                SQ2 = v3.tile([P, R, 3], f32)
                T1 = v3.tile([P, R, 3], f32)
                W = v3.tile([P, R, 3], f32)
                E1 = v3.tile([P, R, 3], f32)
                E2 = v3.tile([P, R, 3], f32)
                TN = v3.tile([P, R, 3], f32)
                TO = v3.tile([P, R, 3], f32)
                S1 = sc.tile([P, R], f32)
                S1e = sc.tile([P, R], f32)
                IS1 = sc.tile([P, R], f32)
                RS1 = sc.tile([P, R], f32)
                DOT = sc.tile([P, R], f32)
                G = sc.tile([P, R], f32)
                S2 = sc.tile([P, R], f32)
                Q2 = sc.tile([P, R], f32)
                RS2 = sc.tile([P, R], f32)

                # |d1|^2 + eps^2 and its exact reciprocal (cancellation path)
                nc.vector.tensor_sub(D1, Ca, CAa)
                nc.vector.tensor_mul(SQ, D1, D1)
                nc.vector.reduce_sum(out=S1, in_=SQ, axis=AX)
                nc.scalar.activation(out=S1e, in_=S1, func=IDENT, bias=eps)
                nc.vector.reciprocal(out=IS1, in_=S1e)
                # rs1 = rsqrt(|d1|^2+eps^2), only scales outputs -> table ok
                nc.scalar.activation(out=RS1, in_=IS1, func=SQRT, bias=zero)

                # w = v - ((v.d1) * is1) d1
                nc.vector.tensor_sub(V, Na, CAa)
                nc.vector.tensor_mul(P2, V, D1)
                nc.vector.reduce_sum(out=DOT, in_=P2, axis=AX)
                nc.vector.tensor_mul(G, DOT, IS1)
                nc.vector.tensor_mul(T1, D1, bcast(G))
                nc.vector.tensor_sub(W, V, T1)

                # rs2 = rsqrt(|w|^2 + eps^2), only scales outputs
                nc.scalar.activation(out=SQ2, in_=W, func=SQUARE, bias=zero)
                nc.vector.reduce_sum(out=S2, in_=SQ2, axis=AX)
                nc.scalar.activation(out=Q2, in_=S2, func=SQRT, bias=eps)
                nc.vector.reciprocal(out=RS2, in_=Q2)

                nc.vector.tensor_mul(E1, D1, bcast(RS1))
                nc.vector.tensor_mul(E2, W, bcast(RS2))

                # out_C = 1.526*e1 + CA
                nc.vector.scalar_tensor_tensor(
                    out=Ca, in0=E1, scalar=1.526, in1=CAa, op0=MUL, op1=ADD)
                # out_N = -0.525*e1 + (1.363*e2 + CA)
                nc.vector.scalar_tensor_tensor(
                    out=TN, in0=E2, scalar=1.363, in1=CAa, op0=MUL, op1=ADD)
                nc.vector.scalar_tensor_tensor(
                    out=Na, in0=E1, scalar=-0.525, in1=TN, op0=MUL, op1=ADD)
                # out_O = 2.153*e1 + (-1.062*e2 + CA)
                nc.vector.scalar_tensor_tensor(
                    out=TO, in0=E2, scalar=-1.062, in1=CAa, op0=MUL, op1=ADD)
                nc.vector.scalar_tensor_tensor(
                    out=Oa, in0=E1, scalar=2.153, in1=TO, op0=MUL, op1=ADD)

                nc.sync.dma_start(out=Y[i], in_=T)
    nc.finalize()
    return nc


def _get_nc():
    global _NC
    if _NC is None:
        _NC = _build_nc()
    return _NC


def kernel(X, batch_ids=None, max_len=None, **_unused):
    from concourse.bass_utils import run_bass_kernel_spmd

    X = np.ascontiguousarray(np.asarray(X, dtype=np.float32))
    assert X.shape == (N_TOTAL, 14, 3), X.shape
    nc = _get_nc()
    shards = X.reshape(N_CORES, TILES, P, R, C42)
    in_maps = [{"X": shards[c]} for c in range(N_CORES)]
    res = run_bass_kernel_spmd(nc, in_maps, list(range(N_CORES))).results
    out = np.stack([res[c]["Y"] for c in range(N_CORES)])
    return out.reshape(N_TOTAL, 14, 3)


# revision 8
# speedup vs baseline: 1.0124x; 1.0124x over previous
"""Trainium2 Bass kernel for nn_BackboneModel (backbone frame rebuild).

The reference scatters rows into a padded [B, L, 14, 3] block, builds
Gram-Schmidt rigid frames from (N, CA, C), places ideal N/CA/C/O atoms,
and gathers the valid rows back.  Scatter followed by gather at the same
(batch_id, pos) indices is an identity permutation over the valid rows,
so the whole model is a pure per-row function of X[i]:

    e1 = normalize(C - CA)                      (normalize: v * rsqrt(|v|^2 + eps^2))
    e2 = normalize((N - CA) - ((N - CA).e1) e1)
    out[0] = -0.525*e1 + 1.363*e2 + CA          (N)
    out[1] = CA                                 (CA)
    out[2] =  1.526*e1            + CA          (C)
    out[3] =  2.153*e1 - 1.062*e2 + CA          (O)
    out[4:14] = X[4:14]                         (passthrough)

(X_IDEAL has z == 0 for all four atoms, so e3 = e1 x e2 is never needed,
and batch_ids never affects output values.)

Numerics: the Gram-Schmidt rejection w = v - (v.e1)e1 suffers catastrophic
cancellation, which amplifies any error in e1 by ~|v|/|w| (observed 250x).
The ACT-engine Sqrt is table-based (~7e-6 rel), so e1 via sqrt+reciprocal
is not accurate enough for that path.  Instead the rejection uses the exact
DVE reciprocal:  w = v - ((v.d1) / (|d1|^2 + eps^2)) d1,  and the table
sqrt is only used for the final normalize scalars, where its error is not
amplified.  Measured absmax vs the f32 jax reference: ~5e-5.

Sharding: data-parallel, 8 equal contiguous row chunks of 98304 rows.
Each core processes its chunk as 6 tiles of [128 partitions x 128 rows x 42 f32],
computing in place in the loaded tile so both the load and the store are a
single fully-contiguous ~2.75 MB DMA per tile.
"""

import numpy as np

N_CORES = 8
N_TOTAL = 786432
N_CORE = N_TOTAL // N_CORES      # 98304 rows per core
P = 128                          # SBUF partitions
R = 128                          # rows per partition per tile
ROWS_PER_TILE = P * R            # 16384
TILES = N_CORE // ROWS_PER_TILE  # 6
C42 = 42                         # 14 atoms * 3 coords
EPS2 = 1e-6                      # FrameBuilder distance_eps squared

_NC = None


def _build_nc():
    import concourse.bacc as bacc
    import concourse.tile as tile
    from concourse import mybir

    f32 = mybir.dt.float32
    AX = mybir.AxisListType.X
    MUL = mybir.AluOpType.mult
    ADD = mybir.AluOpType.add
    SQRT = mybir.ActivationFunctionType.Sqrt
    SQUARE = mybir.ActivationFunctionType.Square
    IDENT = mybir.ActivationFunctionType.Identity

    nc = bacc.Bacc()
    X = nc.declare_dram_parameter("X", [TILES, P, R, C42], f32, isOutput=False)
    Y = nc.declare_dram_parameter("Y", [TILES, P, R, C42], f32, isOutput=True)

    def bcast(s):  # [P, R] per-row scalar -> [P, R, 3]
        return s[:, :, None].broadcast_to([P, R, 3])

    with tile.TileContext(nc) as tc:
        with tc.tile_pool(name="io", bufs=4) as io, \
             tc.tile_pool(name="v3", bufs=2) as v3, \
             tc.tile_pool(name="sc", bufs=2) as sc, \
             tc.tile_pool(name="one", bufs=1) as one:
            eps = one.tile([P, 1], f32)
            nc.vector.memset(eps, EPS2)
            zero = one.tile([P, 1], f32)
            nc.vector.memset(zero, 0.0)

            for i in range(TILES):
                T = io.tile([P, R, C42], f32)
                nc.sync.dma_start(out=T, in_=X[i])

                Na = T[:, :, 0:3]    # N  (input; overwritten with out_N)
                CAa = T[:, :, 3:6]   # CA (unchanged -> out_CA)
                Ca = T[:, :, 6:9]    # C  (input; overwritten with out_C)
                Oa = T[:, :, 9:12]   # O  (input unused; overwritten with out_O)

                D1 = v3.tile([P, R, 3], f32)
                V = v3.tile([P, R, 3], f32)
                SQ = v3.tile([P, R, 3], f32)
                P2 = v3.tile([P, R, 3], f32)
                SQ2 = v3.tile([P, R, 3], f32)
                T1 = v3.tile([P, R, 3], f32)
                W = v3.tile([P, R, 3], f32)
                E1 = v3.tile([P, R, 3], f32)
                E2 = v3.tile([P, R, 3], f32)
                TN = v3.tile([P, R, 3], f32)
                TO = v3.tile([P, R, 3], f32)
                S1 = sc.tile([P, R], f32)
                SCR = sc.tile([P, R], f32)
                S1e = sc.tile([P, R], f32)
                IS1 = sc.tile([P, R], f32)
                RS1 = sc.tile([P, R], f32)
                DOT = sc.tile([P, R], f32)
                G = sc.tile([P, R], f32)
                S2 = sc.tile([P, R], f32)
                Q2 = sc.tile([P, R], f32)
                RS2 = sc.tile([P, R], f32)

                # |d1|^2 + eps^2 and its exact reciprocal (cancellation path)
                nc.gpsimd.tensor_sub(D1, Ca, CAa)
                nc.vector.tensor_mul(SQ, D1, D1)
                nc.vector.reduce_sum(out=S1, in_=SQ, axis=AX)
                nc.scalar.activation(out=S1e, in_=S1, func=IDENT, bias=eps)
                nc.vector.reciprocal_approx_accurate(out=IS1, in_=S1e, scratch=SCR)
                # rs1 = rsqrt(|d1|^2+eps^2), only scales outputs -> table ok
                nc.scalar.activation(out=RS1, in_=IS1, func=SQRT, bias=zero)

                # w = v - ((v.d1) * is1) d1
                nc.gpsimd.tensor_sub(V, Na, CAa)
                nc.vector.tensor_mul(P2, V, D1)
                nc.vector.reduce_sum(out=DOT, in_=P2, axis=AX)
                nc.vector.tensor_mul(G, DOT, IS1)
                nc.vector.tensor_mul(T1, D1, bcast(G))
                nc.vector.tensor_sub(W, V, T1)

                # rs2 = rsqrt(|w|^2 + eps^2), only scales outputs
                nc.scalar.activation(out=SQ2, in_=W, func=SQUARE, bias=zero)
                nc.vector.reduce_sum(out=S2, in_=SQ2, axis=AX)
                nc.scalar.activation(out=Q2, in_=S2, func=SQRT, bias=eps)
                nc.vector.reciprocal_approx_fast(out=RS2, in_=Q2)

                nc.vector.tensor_mul(E1, D1, bcast(RS1))
                nc.vector.tensor_mul(E2, W, bcast(RS2))

                # out_C = 1.526*e1 + CA
                nc.vector.scalar_tensor_tensor(
                    out=Ca, in0=E1, scalar=1.526, in1=CAa, op0=MUL, op1=ADD)
                # out_N = -0.525*e1 + (1.363*e2 + CA)
                nc.vector.scalar_tensor_tensor(
                    out=TN, in0=E2, scalar=1.363, in1=CAa, op0=MUL, op1=ADD)
                nc.vector.scalar_tensor_tensor(
                    out=Na, in0=E1, scalar=-0.525, in1=TN, op0=MUL, op1=ADD)
                # out_O = 2.153*e1 + (-1.062*e2 + CA)
                nc.vector.scalar_tensor_tensor(
                    out=TO, in0=E2, scalar=-1.062, in1=CAa, op0=MUL, op1=ADD)
                nc.vector.scalar_tensor_tensor(
                    out=Oa, in0=E1, scalar=2.153, in1=TO, op0=MUL, op1=ADD)

                nc.scalar.dma_start(out=Y[i], in_=T)
    nc.finalize()
    return nc


def _get_nc():
    global _NC
    if _NC is None:
        _NC = _build_nc()
    return _NC


def kernel(X, batch_ids=None, max_len=None, **_unused):
    from concourse.bass_utils import run_bass_kernel_spmd

    X = np.ascontiguousarray(np.asarray(X, dtype=np.float32))
    assert X.shape == (N_TOTAL, 14, 3), X.shape
    nc = _get_nc()
    shards = X.reshape(N_CORES, TILES, P, R, C42)
    in_maps = [{"X": shards[c]} for c in range(N_CORES)]
    res = run_bass_kernel_spmd(nc, in_maps, list(range(N_CORES))).results
    out = np.stack([res[c]["Y"] for c in range(N_CORES)])
    return out.reshape(N_TOTAL, 14, 3)


# revision 9
# speedup vs baseline: 1.1233x; 1.1096x over previous
"""Trainium2 Bass kernel for nn_BackboneModel (backbone frame rebuild).

The reference scatters rows into a padded [B, L, 14, 3] block, builds
Gram-Schmidt rigid frames from (N, CA, C), places ideal N/CA/C/O atoms,
and gathers the valid rows back.  Scatter followed by gather at the same
(batch_id, pos) indices is an identity permutation over the valid rows,
so the whole model is a pure per-row function of X[i]:

    e1 = normalize(C - CA)                      (normalize: v * rsqrt(|v|^2 + eps^2))
    e2 = normalize((N - CA) - ((N - CA).e1) e1)
    out[0] = -0.525*e1 + 1.363*e2 + CA          (N)
    out[1] = CA                                 (CA)
    out[2] =  1.526*e1            + CA          (C)
    out[3] =  2.153*e1 - 1.062*e2 + CA          (O)
    out[4:14] = X[4:14]                         (passthrough)

(X_IDEAL has z == 0 for all four atoms, so e3 = e1 x e2 is never needed,
and batch_ids never affects output values.)

Numerics: the Gram-Schmidt rejection w = v - (v.e1)e1 suffers catastrophic
cancellation, which amplifies any error in e1 by ~|v|/|w| (observed 250x).
The ACT-engine Sqrt is table-based (~7e-6 rel), so e1 via sqrt+reciprocal
is not accurate enough for that path.  Instead the rejection uses the exact
DVE reciprocal:  w = v - ((v.d1) / (|d1|^2 + eps^2)) d1,  and the table
sqrt is only used for the final normalize scalars, where its error is not
amplified.  Measured absmax vs the f32 jax reference: ~5e-5.

Sharding: data-parallel, 8 equal contiguous row chunks of 98304 rows.
Each core processes its chunk as 6 tiles of [128 partitions x 128 rows x 42 f32],
computing in place in the loaded tile so both the load and the store are a
single fully-contiguous ~2.75 MB DMA per tile.
"""

import numpy as np

N_CORES = 8
N_TOTAL = 786432
N_CORE = N_TOTAL // N_CORES      # 98304 rows per core
P = 128                          # SBUF partitions
R = 128                          # rows per partition per tile
ROWS_PER_TILE = P * R            # 16384
TILES = N_CORE // ROWS_PER_TILE  # 6
C42 = 42                         # 14 atoms * 3 coords
EPS2 = 1e-6                      # FrameBuilder distance_eps squared

_NC = None


def _build_nc():
    import concourse.bacc as bacc
    import concourse.tile as tile
    from concourse import mybir

    f32 = mybir.dt.float32
    AX = mybir.AxisListType.X
    MUL = mybir.AluOpType.mult
    ADD = mybir.AluOpType.add
    SQRT = mybir.ActivationFunctionType.Sqrt
    SQUARE = mybir.ActivationFunctionType.Square
    IDENT = mybir.ActivationFunctionType.Identity

    nc = bacc.Bacc()
    X = nc.declare_dram_parameter("X", [TILES, P, R, C42], f32, isOutput=False)
    Y = nc.declare_dram_parameter("Y", [TILES, P, R, C42], f32, isOutput=True)

    def bcast(s):  # [P, R] per-row scalar -> [P, R, 3]
        return s[:, :, None].broadcast_to([P, R, 3])

    with tile.TileContext(nc) as tc:
        with tc.tile_pool(name="io", bufs=5) as io, \
             tc.tile_pool(name="v3", bufs=2) as v3, \
             tc.tile_pool(name="sc", bufs=2) as sc, \
             tc.tile_pool(name="one", bufs=1) as one:
            eps = one.tile([P, 1], f32)
            nc.vector.memset(eps, EPS2)
            zero = one.tile([P, 1], f32)
            nc.vector.memset(zero, 0.0)

            for i in range(TILES):
                T = io.tile([P, R, C42], f32)
                nc.sync.dma_start(out=T, in_=X[i])

                Na = T[:, :, 0:3]    # N  (input; overwritten with out_N)
                CAa = T[:, :, 3:6]   # CA (unchanged -> out_CA)
                Ca = T[:, :, 6:9]    # C  (input; overwritten with out_C)
                Oa = T[:, :, 9:12]   # O  (input unused; overwritten with out_O)

                D1 = v3.tile([P, R, 3], f32)
                V = v3.tile([P, R, 3], f32)
                SQ = v3.tile([P, R, 3], f32)
                P2 = v3.tile([P, R, 3], f32)
                SQ2 = v3.tile([P, R, 3], f32)
                T1 = v3.tile([P, R, 3], f32)
                W = v3.tile([P, R, 3], f32)
                E1 = v3.tile([P, R, 3], f32)
                E2 = v3.tile([P, R, 3], f32)
                TN = v3.tile([P, R, 3], f32)
                TO = v3.tile([P, R, 3], f32)
                S1 = sc.tile([P, R], f32)
                SCR = sc.tile([P, R], f32)
                S1e = sc.tile([P, R], f32)
                IS1 = sc.tile([P, R], f32)
                RS1 = sc.tile([P, R], f32)
                DOT = sc.tile([P, R], f32)
                G = sc.tile([P, R], f32)
                S2 = sc.tile([P, R], f32)
                Q2 = sc.tile([P, R], f32)
                RS2 = sc.tile([P, R], f32)

                # |d1|^2 + eps^2 and its exact reciprocal (cancellation path)
                nc.vector.tensor_sub(D1, Ca, CAa)
                nc.vector.tensor_mul(SQ, D1, D1)
                nc.vector.reduce_sum(out=S1, in_=SQ, axis=AX)
                nc.scalar.activation(out=S1e, in_=S1, func=IDENT, bias=eps)
                nc.vector.reciprocal_approx_accurate(out=IS1, in_=S1e, scratch=SCR)
                # rs1 = rsqrt(|d1|^2+eps^2), only scales outputs -> table ok
                nc.scalar.activation(out=RS1, in_=IS1, func=SQRT, bias=zero)

                # w = v - ((v.d1) * is1) d1
                nc.vector.tensor_sub(V, Na, CAa)
                nc.vector.tensor_mul(P2, V, D1)
                nc.vector.reduce_sum(out=DOT, in_=P2, axis=AX)
                nc.vector.tensor_mul(G, DOT, IS1)
                nc.vector.tensor_mul(T1, D1, bcast(G))
                nc.vector.tensor_sub(W, V, T1)

                # rs2 = rsqrt(|w|^2 + eps^2), only scales outputs
                nc.scalar.activation(out=SQ2, in_=W, func=SQUARE, bias=zero)
                nc.vector.reduce_sum(out=S2, in_=SQ2, axis=AX)
                nc.scalar.activation(out=Q2, in_=S2, func=SQRT, bias=eps)
                nc.vector.reciprocal_approx_fast(out=RS2, in_=Q2)

                nc.vector.tensor_mul(E1, D1, bcast(RS1))
                nc.vector.tensor_mul(E2, W, bcast(RS2))

                # out_C = 1.526*e1 + CA
                nc.vector.scalar_tensor_tensor(
                    out=Ca, in0=E1, scalar=1.526, in1=CAa, op0=MUL, op1=ADD)
                # out_N = -0.525*e1 + (1.363*e2 + CA)
                nc.vector.scalar_tensor_tensor(
                    out=TN, in0=E2, scalar=1.363, in1=CAa, op0=MUL, op1=ADD)
                nc.vector.scalar_tensor_tensor(
                    out=Na, in0=E1, scalar=-0.525, in1=TN, op0=MUL, op1=ADD)
                # out_O = 2.153*e1 + (-1.062*e2 + CA)
                nc.vector.scalar_tensor_tensor(
                    out=TO, in0=E2, scalar=-1.062, in1=CAa, op0=MUL, op1=ADD)
                nc.vector.scalar_tensor_tensor(
                    out=Oa, in0=E1, scalar=2.153, in1=TO, op0=MUL, op1=ADD)

                nc.gpsimd.dma_start(out=Y[i], in_=T)
    nc.finalize()
    return nc


def _get_nc():
    global _NC
    if _NC is None:
        _NC = _build_nc()
    return _NC


def kernel(X, batch_ids=None, max_len=None, **_unused):
    from concourse.bass_utils import run_bass_kernel_spmd

    X = np.ascontiguousarray(np.asarray(X, dtype=np.float32))
    assert X.shape == (N_TOTAL, 14, 3), X.shape
    nc = _get_nc()
    shards = X.reshape(N_CORES, TILES, P, R, C42)
    in_maps = [{"X": shards[c]} for c in range(N_CORES)]
    res = run_bass_kernel_spmd(nc, in_maps, list(range(N_CORES))).results
    out = np.stack([res[c]["Y"] for c in range(N_CORES)])
    return out.reshape(N_TOTAL, 14, 3)
